# revision 1
# baseline (speedup 1.0000x reference)
"""DenseCapsule routing (2 iterations) on 8 Trainium2 cores.

Sharding: caps_in (C=2048) split across 8 cores (256 each); W-shard +
x-shard stay resident in SBUF, u is recomputed on the fly per c-tile.
Routing state is reduced across cores with two 128KB AllReduces.

Math (ITERATIONS=2, v0=0 => logits after iter1 are 0, cc1 = 1/K):
  u[b,k,c,i]   = sum_j W[k,c,i,j] x[b,c,j]
  v1           = squash(sum_c u / K)
  a[b,k,c]     = sum_i u[b,k,c,i] v1[b,k,i]        (logits for iter 2)
  cc           = softmax_k(a)
  v2           = squash(sum_c cc[b,k,c] u[b,k,c,i])   -> output

Per-core layouts (host-prepped):
  xt  [(c,j)=2048, b=64]            pass-1 lhsT
  wt  [(c,j)=2048, (k,i)=512]       pass-1 rhs & pass-2 u-matmul rhs
  xdo [g=16, (c'16,j8)=128, oct=8, (c16,b8)=128]
      block-diag x: xdo[g,(c'j),o,(c,b)] = x[o*8+b, c0+16g+c', j] * (c==c')
      pass-2 u-matmul lhsT -> psum_u[(c,b), (k,i)] = u[b,k,c,i]
  obd [(c16,b'8)=128, oct=8, b=64]  ones block-diag: delta(b == o*8+b')
      s2p reduction lhsT: psum_s2[b,(k,i)] += sum_c tmp2[(c,b'),(k,i)]
  sel [b'=64, oct=8, (c16,b8)=128]  v1 replication lhsT: delta(b' == o*8+b)
"""

import numpy as np

import concourse.bacc as bacc
import concourse.bass as bass
import concourse.tile as tile
from concourse import mybir
from concourse._compat import with_exitstack
from concourse.bass_utils import run_bass_kernel_spmd

NC = 8
B = 64
C = 2048
J = 8
K = 32
I = 16
CL = C // NC        # 256 local caps_in
G = CL // 16        # 16 c-tiles (16 c's each -> 128 (c,j) rows)
KI = K * I          # 512
EPS = 1e-7

F32 = mybir.dt.float32
F32R = mybir.dt.float32r
BF16 = mybir.dt.bfloat16

TRACE = False           # test.py sets True to capture NTFF timing
LAST_RESULTS = None     # BassKernelResults of the last run


def _bcast_last(ap, n):
    """Append a stride-0 dim of size n to an AP (free-dim broadcast)."""
    return bass.AP(tensor=ap.tensor, offset=ap.offset, ap=[*ap.ap, [0, n]])


def _bcast_mid(ap, n):
    """Insert a stride-0 dim of size n before the last free dim... actually
    appends like _bcast_last; kept separate for call-site clarity when the
    broadcast dim is the innermost of the output."""
    return bass.AP(tensor=ap.tensor, offset=ap.offset, ap=[*ap.ap, [0, n]])


def _squash(nc, pool, eps_t, s_sb, pre, out_dt=F32):
    """v = squash(pre * s_sb) for s_sb [B, KI] f32, squash over i (last 16).

    squash(s) = (|s|^2 / (1 + |s|^2)) * s / sqrt(|s|^2 + EPS), per (b, k).
    Returns [B, KI] tile of out_dt.
    """
    sq = pool.tile([B, K, I], F32, tag="sq_sq")
    s3 = s_sb[:].rearrange("p (k i) -> p k i", i=I)
    nc.vector.tensor_mul(sq[:], s3, s3)
    n0 = pool.tile([B, K], F32, tag="sq_n0")
    nc.vector.reduce_sum(n0[:], sq[:], axis=mybir.AxisListType.X)
    sn = pool.tile([B, K], F32, tag="sq_sn")
    nc.scalar.mul(sn[:], n0[:], pre * pre)          # |s|^2
    rt = pool.tile([B, K], F32, tag="sq_rt")
    nc.scalar.activation(rt[:], sn[:], mybir.ActivationFunctionType.Sqrt,
                         bias=eps_t[:], scale=1.0)  # sqrt(|s|^2 + eps)
    dn = pool.tile([B, K], F32, tag="sq_dn")
    nc.scalar.add(dn[:], sn[:], 1.0)                # 1 + |s|^2
    dd = pool.tile([B, K], F32, tag="sq_dd")
    nc.vector.tensor_mul(dd[:], dn[:], rt[:])
    rc = pool.tile([B, K], F32, tag="sq_rc")
    nc.vector.reciprocal(rc[:], dd[:])
    f0 = pool.tile([B, K], F32, tag="sq_f0")
    nc.vector.tensor_mul(f0[:], sn[:], rc[:])
    g0 = pool.tile([B, K], F32, tag="sq_g0")
    nc.scalar.mul(g0[:], f0[:], pre)                # scale applied to raw s_sb
    v = pool.tile([B, K, I], out_dt, tag="sq_v")
    nc.vector.tensor_mul(v[:], s3, _bcast_last(g0[:], I))
    return v


@with_exitstack
def _body(ctx, tc, xt, wt, xdo, obd, sel, out_d):
    nc = tc.nc
    singles = ctx.enter_context(tc.tile_pool(name="singles", bufs=1))
    psA = ctx.enter_context(tc.tile_pool(name="psA", bufs=1, space="PSUM"))
    psU = ctx.enter_context(tc.tile_pool(name="psU", bufs=3, space="PSUM"))
    work = ctx.enter_context(tc.tile_pool(name="work", bufs=2))
    upool = ctx.enter_context(tc.tile_pool(name="upool", bufs=8))
    sm = ctx.enter_context(tc.tile_pool(name="sm", bufs=2))
    dram = ctx.enter_context(tc.tile_pool(name="dram", bufs=1, space="DRAM"))
    ar1_in = dram.tile([B, KI], F32, name="ar1_in")
    ar1_out = dram.tile([B, KI], F32, name="ar1_out", addr_space="Shared")
    ar2_in = dram.tile([B, KI], F32, name="ar2_in")
    ar2_out = dram.tile([B, KI], F32, name="ar2_out", addr_space="Shared")

    # ---- resident inputs (one tile per DMA so consumers wait on 1 sem) ----
    xt_sb = [singles.tile([128, B], BF16, name=f"xt{g}", tag=f"xt{g}") for g in range(G)]
    wt_sb = [singles.tile([128, KI], BF16, name=f"wt{g}", tag=f"wt{g}") for g in range(G)]
    xdo_sb = [singles.tile([128, 8, 128], BF16, name=f"xdo{g}", tag=f"xdo{g}") for g in range(G)]
    obd_sb = singles.tile([128, 8, B], BF16)
    sel_sb = singles.tile([B, 8, 128], BF16)
    for g in range(G):
        nc.sync.dma_start(out=xt_sb[g][:], in_=xt[g * 128:(g + 1) * 128, :])
        nc.sync.dma_start(out=wt_sb[g][:], in_=wt[g * 128:(g + 1) * 128, :])
    nc.sync.dma_start(out=obd_sb[:], in_=obd)
    nc.sync.dma_start(out=sel_sb[:], in_=sel)
    for g in range(G):
        nc.sync.dma_start(out=xdo_sb[g][:], in_=xdo[g])
    eps_t = singles.tile([B, 1], F32)
    nc.vector.memset(eps_t[:], EPS)

    # ---- pass 1: s1 partial = sum_{c local, j} W x ----
    ps_s1 = psA.tile([B, KI], F32)
    for g in range(G):
        nc.tensor.matmul(ps_s1[:], lhsT=xt_sb[g][:],
                         rhs=wt_sb[g][:],
                         start=(g == 0), stop=(g == G - 1))
    # ---- pass 2, software-pipelined: produce u tiles (PE+ACT), consume
    # (DVE routing chain) once v1 is ready.  PRO tiles are produced before
    # the AllReduce so PE/ACT fill the collective latency.
    ps_s2 = psA.tile([B, KI], F32)
    nmm = 8 * G
    it = 0
    PRO = 7
    u_tiles = {}

    def produce(g):
        u_g = upool.tile([128, 8, KI], BF16, name=f"ug{g}", tag="ug")
        for op in range(4):
            ps_u = psU.tile([128, 2, KI], F32, tag="psu")
            for h in range(2):
                nc.tensor.matmul(ps_u[:, h, :], lhsT=xdo_sb[g][:, 2 * op + h, :],
                                 rhs=wt_sb[g][:],
                                 start=True, stop=True)
            nc.scalar.copy(u_g[:, 2 * op:2 * op + 2, :], ps_u[:])
        u_tiles[g] = u_g

    def consume(g):
        nonlocal it
        u_g = u_tiles.pop(g)
        tmp = work.tile([128, 8, KI], BF16, tag="tmp")
        nc.vector.tensor_mul(tmp[:], u_g[:], v1rep[:])
        t4 = tmp[:].rearrange("p o (k i) -> p o k i", i=I)
        f1 = work.tile([128, 8, K, 8], BF16, tag="f1")
        nc.vector.tensor_add(f1[:], t4[:, :, :, 0:8], t4[:, :, :, 8:16])
        f2 = sm.tile([128, 8, K, 4], BF16, tag="f2")
        nc.vector.tensor_add(f2[:], f1[:, :, :, 0:4], f1[:, :, :, 4:8])
        f3 = sm.tile([128, 8, K, 2], BF16, tag="f3")
        nc.vector.tensor_add(f3[:], f2[:, :, :, 0:2], f2[:, :, :, 2:4])
        a_t = sm.tile([128, 8, K], F32, tag="a")
        nc.vector.tensor_add(a_t[:], f3[:, :, :, 0], f3[:, :, :, 1])
        e_t = sm.tile([128, 8, K], F32, tag="e")
        nc.scalar.activation(e_t[:], a_t[:],
                             mybir.ActivationFunctionType.Exp, scale=1.0)
        den = sm.tile([128, 8], F32, tag="den")
        nc.vector.reduce_sum(den[:], e_t[:], axis=mybir.AxisListType.X)
        rcp = sm.tile([128, 8], F32, tag="rcp")
        nc.vector.reciprocal(rcp[:], den[:])
        cc = sm.tile([128, 8, K], BF16, tag="cc")
        nc.vector.tensor_mul(cc[:], e_t[:], _bcast_mid(rcp[:], K))
        tmp2 = work.tile([128, 8, K, I], BF16, tag="tmp2")
        nc.vector.tensor_mul(
            tmp2[:], u_g[:].rearrange("p o (k i) -> p o k i", i=I),
            _bcast_last(cc[:], I))
        for o in range(8):
            nc.tensor.matmul(ps_s2[:], lhsT=obd_sb[:, o, :],
                             rhs=tmp2[:, o, :, :].rearrange("p k i -> p (k i)"),
                             start=(it == 0), stop=(it == nmm - 1))
            it += 1

    for g in range(PRO):
        produce(g)

    # ---- AllReduce s1, v1 = squash(s1/K), replicate across partitions ----
    s1p = sm.tile([B, KI], F32, tag="s1p")
    nc.scalar.copy(s1p[:], ps_s1[:])
    nc.sync.dma_start(out=ar1_in[:], in_=s1p[:])
    nc.gpsimd.collective_compute(
        "AllReduce", mybir.AluOpType.add,
        replica_groups=[list(range(NC))], ins=[ar1_in.opt()], outs=[ar1_out.opt()])
    s1 = sm.tile([B, KI], F32, tag="s1")
    nc.sync.dma_start(out=s1[:], in_=ar1_out[:])
    v1 = _squash(nc, sm, eps_t, s1, 1.0 / K)
    v1b = sm.tile([B, KI], BF16, tag="v1b")
    nc.vector.tensor_copy(v1b[:], v1[:].rearrange("p k i -> p (k i)"))
    v1rep = singles.tile([128, 8, KI], BF16)
    v1d = dram.tile([B, KI], BF16, name="v1d")
    nc.sync.dma_start(out=v1d[:], in_=v1b[:])
    v1d_ap = v1d[:]
    for o in range(8):
        src_ap = bass.AP(tensor=v1d_ap.tensor,
                         offset=v1d_ap.offset + o * 8 * KI,
                         ap=[[0, 16], [KI, 8], [1, KI]])
        nc.sync.dma_start(out=v1rep[:, o, :], in_=src_ap)

    for g in range(PRO, G):
        produce(g)
        consume(g - PRO)
    for g in range(G - PRO, G):
        consume(g)

    s2p = sm.tile([B, KI], F32, tag="s2p")
    nc.scalar.copy(s2p[:], ps_s2[:])
    nc.sync.dma_start(out=ar2_in[:], in_=s2p[:])
    nc.gpsimd.collective_compute(
        "AllReduce", mybir.AluOpType.add,
        replica_groups=[list(range(NC))], ins=[ar2_in.opt()], outs=[ar2_out.opt()])
    s2 = sm.tile([B, KI], F32, tag="s2")
    nc.sync.dma_start(out=s2[:], in_=ar2_out[:])
    v2 = _squash(nc, sm, eps_t, s2, 1.0)
    nc.sync.dma_start(out=out_d, in_=v2[:].rearrange("p k i -> p (k i)"))


_PROG = None


def _get_program():
    global _PROG
    if _PROG is None:
        nc = bacc.Bacc("TRN2", target_bir_lowering=False, debug=False,
                       num_devices=NC)
        xt_d = nc.dram_tensor("xt", [CL * J, B], BF16, kind="ExternalInput")
        wt_d = nc.dram_tensor("wt", [CL * J, KI], BF16, kind="ExternalInput")
        xdo_d = nc.dram_tensor("xdo", [G, 128, 8, 128], BF16,
                               kind="ExternalInput")
        obd_d = nc.dram_tensor("obd", [128, 8, B], BF16, kind="ExternalInput")
        sel_d = nc.dram_tensor("sel", [B, 8, 128], BF16, kind="ExternalInput")
        out_d = nc.dram_tensor("out", [B, KI], F32, kind="ExternalOutput")
        with tile.TileContext(nc) as tc:
            _body(tc, xt_d[:], wt_d[:], xdo_d[:], obd_d[:], sel_d[:],
                  out_d[:])
        nc.compile()
        _PROG = nc
    return _PROG


def _constant_mats():
    import ml_dtypes
    obd = np.zeros((16, 8, 8, B), np.float32)       # [c, b', oct, b]
    for o in range(8):
        for bp in range(8):
            obd[:, bp, o, o * 8 + bp] = 1.0
    obd = obd.reshape(128, 8, B).astype(ml_dtypes.bfloat16)
    sel = np.zeros((B, 8, 16, 8), np.float32)       # [b', oct, c, b]
    for o in range(8):
        for b in range(8):
            sel[o * 8 + b, o, :, b] = 1.0
    sel = sel.reshape(B, 8, 128).astype(ml_dtypes.bfloat16)
    return obd, sel


def kernel(x, W):
    global LAST_RESULTS
    x = np.ascontiguousarray(np.asarray(x, np.float32))
    W = np.ascontiguousarray(np.asarray(W, np.float32))
    assert x.shape == (B, C, J) and W.shape == (K, C, I, J)
    nc = _get_program()
    obd, sel = _constant_mats()
    in_maps = []
    for m in range(NC):
        xs = x[:, m * CL:(m + 1) * CL, :]                       # [B, CL, J]
        Ws = W[:, m * CL:(m + 1) * CL, :, :]                    # [K, CL, I, J]
        import ml_dtypes
        bf = ml_dtypes.bfloat16
        xt = np.ascontiguousarray(
            xs.transpose(1, 2, 0)).reshape(CL * J, B).astype(bf)
        wt = np.ascontiguousarray(
            Ws.transpose(1, 3, 0, 2)).reshape(CL * J, KI).astype(bf)
        A = xs.reshape(8, 8, G, 16, J)                          # [o, b, g, c', j]
        xdo = np.zeros((G, 16, J, 8, 16, 8), np.float32)        # [g,c',j,o,c,b]
        for cp in range(16):
            xdo[:, cp, :, :, cp, :] = A[:, :, :, cp, :].transpose(2, 3, 0, 1)
        xdo = xdo.reshape(G, 128, 8, 128).astype(bf)
        in_maps.append({"xt": xt, "wt": wt, "xdo": xdo,
                        "obd": obd, "sel": sel})
    res = run_bass_kernel_spmd(nc, in_maps, core_ids=list(range(NC)),
                               trace=TRACE)
    LAST_RESULTS = res
    return np.asarray(res.results[0]["out"], np.float32).reshape(B, K, I)



# revision 13
# speedup vs baseline: 1.1127x; 1.1127x over previous
"""DenseCapsule routing (2 iterations) on 8 Trainium2 cores — P-route.

Sharding: caps_in (C=2048) split across 8 cores (256 each).

Math (ITERATIONS=2, v0=0 => logits after iter1 are 0, cc1 = 1/K):
  s1[b,ki]   = sum_{c,j} W x          (pass1 matmul, AllReduce)
  v1         = squash(s1/K)
  P[b,k,c,j] = sum_i v1[b,k,i] W[k,c,i,j]     (PE: v1 block-diag x W2)
  a[b,k,c]   = sum_j x[b,c,j] P[b,k,c,j]      (DVE mult + j-tree)
  e = exp(a);  den[b,c] = sum_k e             (PE: sel2 matmul)
  xden[(c,j),b] = x/den;  y[(c,j),(k,b)] = eT * xden   (DVE)
  s2T[(k8,i),(grp,b)] = diag_k8( wt_grp^T y_grp )      (PE + ACT extract)
  ReduceScatter(s2T) by k8; per-core squash -> out slice; host assembles.

Layouts (free dims j-major for the a-branch, c-major rows for s2):
  wt   [(c,j)=2048 c-major rows, (k,i)=512]   pass1 rhs + s2 lhsT
  w2   [4][128=(p2,k2,i), (j,c)=2048 j-major] P rhs (4 pairs per tile)
  xcb  [(c,j)=2048 c-major, b=64]             pass1 lhsT + xden src
  xrep [128=(k2,b), (j,c)=2048 j-major]       a-branch mult operand
  sel2 [128=(k2,b), 64]  delta(b'=b)          den matmul lhsT
"""

from contextlib import ExitStack

import numpy as np

import concourse.bacc as bacc
import concourse.bass as bass
import concourse.tile as tile
from concourse import mybir
from concourse._compat import with_exitstack
from concourse.bass_utils import run_bass_kernel_spmd

NC = 8
B = 64
C = 2048
J = 8
K = 32
I = 16
CL = C // NC        # 256 local caps_in
G = CL // 16        # 16 (c,j)-row tiles of 128
KI = K * I          # 512
NP = K // 2         # 16 k-pairs
EPS = 1e-7

F32 = mybir.dt.float32
F16 = mybir.dt.float16

TRACE = False
LAST_RESULTS = None


def _ap(t_ap, off, dims):
    return bass.AP(tensor=t_ap.tensor, offset=t_ap.offset + off, ap=dims)


def _bcast(ap, n):
    """Append a stride-0 dim of size n (free-dim broadcast)."""
    return bass.AP(tensor=ap.tensor, offset=ap.offset, ap=[*ap.ap, [0, n]])


@with_exitstack
def _body(ctx, tc, wt, w2, xcb, xrep, sel2, idn, idn32, mask4,
          out_d):
    nc = tc.nc
    singles = ctx.enter_context(tc.tile_pool(name="singles", bufs=1))
    sm = ctx.enter_context(tc.tile_pool(name="sm", bufs=2))
    amp = ctx.enter_context(tc.tile_pool(name="amp", bufs=2))
    ypool = ctx.enter_context(tc.tile_pool(name="ypool", bufs=3))
    psT = ctx.enter_context(tc.tile_pool(name="psT", bufs=2, space="PSUM"))
    dram = ctx.enter_context(tc.tile_pool(name="dram", bufs=1, space="DRAM"))

    ar1_in = dram.tile([B, KI], F32, name="ar1_in")
    ar1_out = dram.tile([B, KI], F32, name="ar1_out", addr_space="Shared")
    eTd = dram.tile([CL, 4 * KI], F16, name="eTd")
    rdTd = dram.tile([CL, B], F16, name="rdTd")
    ar2_in = dram.tile([128, 256], F32, name="ar2_in")
    ar2_out = dram.tile([128, 256], F32, name="ar2_out", addr_space="Shared")

    # ---------------- resident inputs ----------------
    wt_sb = [singles.tile([128, KI], F16, name=f"wt{g}") for g in range(G)]
    w2_sb = [singles.tile([128, 2048], F16, name=f"w2{t}") for t in range(4)]
    xcb_sb = [singles.tile([128, B], F16, name=f"xcb{g}") for g in range(G)]
    xrep_sb = singles.tile([128, 2048], F16, name="xrep")
    sel2_sb = singles.tile([128, B], F16, name="sel2")
    idn_sb = singles.tile([128, 128], F16, name="idn")
    idn32_sb = singles.tile([16, 16], F32, name="idn32")
    for g in range(G):
        nc.sync.dma_start(out=wt_sb[g][:], in_=wt[g * 128:(g + 1) * 128, :])
    for t in range(4):
        nc.sync.dma_start(out=w2_sb[t][:], in_=w2[t])
    for g in range(G):
        nc.sync.dma_start(out=xcb_sb[g][:], in_=xcb[g * 128:(g + 1) * 128, :])
    nc.sync.dma_start(out=xrep_sb[:], in_=xrep)
    nc.sync.dma_start(out=sel2_sb[:], in_=sel2)
    nc.sync.dma_start(out=idn_sb[:], in_=idn)
    nc.sync.dma_start(out=idn32_sb[:], in_=idn32)

    eps_t = singles.tile([B, 1], F32)
    nc.vector.memset(eps_t[:], EPS)
    eps_p = singles.tile([128, 1], F32)
    nc.vector.memset(eps_p[:], EPS)

    # v1 block-diag (filled via mask multiply after v1T is ready)
    v1bd = singles.tile([128, NP * 128], F16, name="v1bd")
    mask4_sb = singles.tile([128, 512], F16, name="mask4")
    nc.sync.dma_start(out=mask4_sb[:], in_=mask4)

    # ---------------- pass 1 + AllReduce ----------------
    with tc.tile_pool(name="psP1", bufs=1, space="PSUM") as psP1:
        ps_s1 = psP1.tile([B, KI], F32)
        for g in range(G):
            nc.tensor.matmul(ps_s1[:], lhsT=xcb_sb[g][:], rhs=wt_sb[g][:],
                             start=(g == 0), stop=(g == G - 1))
        s1p = sm.tile([B, KI], F32, tag="s1p")
        nc.scalar.copy(s1p[:], ps_s1[:])
    nc.sync.dma_start(out=ar1_in[:], in_=s1p[:])
    nc.gpsimd.collective_compute(
        "AllReduce", mybir.AluOpType.add,
        replica_groups=[list(range(NC))], ins=[ar1_in.opt()],
        outs=[ar1_out.opt()])
    s1 = sm.tile([B, KI], F32, tag="s1")
    nc.sync.dma_start(out=s1[:], in_=ar1_out[:])

    # ---------------- squash(s1/K) -> v1b fp16 ----------------
    pre = 1.0 / K
    s3 = s1[:].rearrange("p (k i) -> p k i", i=I)
    sq = sm.tile([B, K, I], F32, tag="sq")
    nc.vector.tensor_mul(sq[:], s3, s3)
    n0 = sm.tile([B, K], F32, tag="n0")
    nc.vector.reduce_sum(n0[:], sq[:], axis=mybir.AxisListType.X)
    sn = sm.tile([B, K], F32, tag="sn")
    nc.scalar.mul(sn[:], n0[:], pre * pre)
    rt = sm.tile([B, K], F32, tag="rt")
    nc.scalar.activation(rt[:], sn[:], mybir.ActivationFunctionType.Sqrt,
                         bias=eps_t[:], scale=1.0)
    dn = sm.tile([B, K], F32, tag="dn")
    nc.scalar.add(dn[:], sn[:], 1.0)
    dd = sm.tile([B, K], F32, tag="dd")
    nc.vector.tensor_mul(dd[:], dn[:], rt[:])
    rc = sm.tile([B, K], F32, tag="rc")
    nc.vector.reciprocal(rc[:], dd[:])
    f0 = sm.tile([B, K], F32, tag="f0")
    nc.vector.tensor_mul(f0[:], n0[:], rc[:])
    g0 = sm.tile([B, K], F32, tag="g0")
    nc.scalar.mul(g0[:], f0[:], pre * pre * pre)
    v1b = sm.tile([B, K, I], F16, tag="v1b")
    nc.vector.tensor_mul(v1b[:], s3, _bcast(g0[:], I))

    # ---------------- v1T via PE transpose; fill v1bd ----------------
    v1T = [singles.tile([128, B], F16, name=f"v1T{t}") for t in range(4)]
    v1f = v1b[:].rearrange("p k i -> p (k i)")
    for t in range(4):
        pt = psT.tile([128, 128], F16, tag="psTh")
        nc.tensor.transpose(pt[:, :B], v1f[:, t * 128:(t + 1) * 128],
                            idn_sb[:B, :B])
        nc.scalar.copy(v1T[t][:], pt[:, :B])
    for t in range(4):
        src0 = bass.AP(tensor=v1T[t][:].tensor, offset=v1T[t][:].offset,
                       ap=[v1T[t][:].ap[0], [0, 8], [1, B]])
        src1 = bass.AP(tensor=mask4_sb[:].tensor, offset=mask4_sb[:].offset,
                       ap=[mask4_sb[:].ap[0], [B, 8], [1, B]])
        dst = v1bd[:, 512 * t:512 * (t + 1)].rearrange(
            "p (q b) -> p q b", b=B)
        nc.vector.tensor_mul(dst, src0, src1)

    # ---------------- phase A: per-pair P -> a -> e ----------------
    eT2 = [singles.tile([128, 4 * KI], F16, name=f"eT2{h}") for h in range(2)]
    eTrep = [singles.tile([128, 2048], F16, name=f"eTrep{g}")
             for g in range(G)]

    with ExitStack() as aes:
        psP = aes.enter_context(
            tc.tile_pool(name="psP", bufs=3, space="PSUM"))
        psDen = aes.enter_context(
            tc.tile_pool(name="psDen", bufs=1, space="PSUM"))
        ps_den = psDen.tile([B, CL], F32)

        def phaseA(p):
            t = p // 4
            Pb = amp.tile([128, 1024], F16, tag="Pb")     # ACT-drained halves
            am = amp.tile([128, 2048], F16, tag="am")
            for q in range(4):
                ps_q = psP.tile([128, 512], F32, tag="psq")
                nc.tensor.matmul(ps_q[:], lhsT=v1bd[:, 128 * p:128 * (p + 1)],
                                 rhs=w2_sb[t][:, q * 512:(q + 1) * 512],
                                 start=True, stop=True)
                sl = slice(q * 512, (q + 1) * 512)
                if q % 2 == 0:
                    nc.scalar.copy(Pb[:, (q // 2) * 512:(q // 2 + 1) * 512],
                                   ps_q[:])
                else:
                    nc.vector.tensor_mul(am[:, sl], ps_q[:], xrep_sb[:, sl])
            for q in (0, 2):
                sl = slice(q * 512, (q + 1) * 512)
                nc.vector.tensor_mul(
                    am[:, sl], Pb[:, (q // 2) * 512:(q // 2 + 1) * 512],
                    xrep_sb[:, sl])
            a3 = am[:].rearrange("p (j c) -> p j c", c=CL)
            t1 = amp.tile([128, 4, CL], F16, tag="t1")
            nc.vector.tensor_add(t1[:], a3[:, 0:4, :], a3[:, 4:8, :])
            t2 = amp.tile([128, 2, CL], F16, tag="t2")
            nc.vector.tensor_add(t2[:], t1[:, 0:2, :], t1[:, 2:4, :])
            a_p = amp.tile([128, CL], F16, tag="a")
            nc.vector.tensor_add(a_p[:], t2[:, 0, :], t2[:, 1, :])
            e_p = amp.tile([128, CL], F16, tag="e")
            nc.scalar.activation(e_p[:], a_p[:],
                                 mybir.ActivationFunctionType.Exp, scale=1.0)
            # den accumulation over pairs
            nc.tensor.matmul(ps_den[:], lhsT=sel2_sb[:], rhs=e_p[:],
                             start=(p == 0), stop=(p == NP - 1))
            # eT: transpose e_p halves into eT2, stage to DRAM
            for h in range(2):
                pt = psT.tile([128, 128], F16, tag="psTh")
                nc.tensor.transpose(pt[:], e_p[:, h * 128:(h + 1) * 128],
                                    idn_sb[:])
                nc.scalar.copy(eT2[h][:, p * 128:(p + 1) * 128], pt[:])
                nc.sync.dma_start(
                    out=_ap(eTd[:], h * 128 * (4 * KI) + p * 128,
                            [[4 * KI, 128], [1, 128]]),
                    in_=eT2[h][:, p * 128:(p + 1) * 128])

        def rep_dmas(grp):
            # replicate eTd rows (c) x8 over j -> eTrep[g][:, grp cols]
            for g in range(G):
                src = _ap(eTd[:], (16 * g) * (4 * KI) + 512 * grp,
                          [[4 * KI, 16], [0, 8], [1, 512]])
                nc.sync.dma_start(
                    out=eTrep[g][:, 512 * grp:512 * (grp + 1)], in_=src)

        for p in range(NP):
            phaseA(p)
            if p % 4 == 3:
                rep_dmas(p // 4)

        # ---------------- den -> xden ----------------
        rcd = sm.tile([B, CL], F32, tag="rcd")
        nc.vector.reciprocal(rcd[:], ps_den[:])
    rch = sm.tile([B, CL], F16, tag="rch")
    nc.scalar.copy(rch[:], rcd[:])
    rdT = [sm.tile([128, B], F16, name=f"rdT{h}") for h in range(2)]
    for h in range(2):
        pt = psT.tile([128, 128], F16, tag="psTh")
        nc.tensor.transpose(pt[:, :B], rch[:, h * 128:(h + 1) * 128],
                            idn_sb[:B, :B])
        nc.scalar.copy(rdT[h][:], pt[:, :B])
        nc.sync.dma_start(out=rdTd[h * 128:(h + 1) * 128, :], in_=rdT[h][:])
    xden = [singles.tile([128, B], F16, name=f"xden{g}") for g in range(G)]
    xdrep = [sm.tile([128, B], F16, name=f"xdr{g}") for g in range(G)]
    for g in range(G):
        src = _ap(rdTd[:], (16 * g) * B, [[B, 16], [0, 8], [1, B]])
        nc.sync.dma_start(out=xdrep[g][:], in_=src)
        nc.vector.tensor_mul(xden[g][:], xcb_sb[g][:], xdrep[g][:])

    # ---------------- phase B: y, s2 matmuls, diag extract ----------------
    with tc.tile_pool(name="psS2", bufs=1, space="PSUM") as psS2:
        ps_s2 = [psS2.tile([128, 512], F32, name=f"pss2{grp}")
                 for grp in range(4)]
        for g in range(G):
            y_g = ypool.tile([128, 2048], F16, tag="y")
            xb = bass.AP(tensor=xden[g][:].tensor,
                         offset=xden[g][:].offset,
                         ap=[xden[g][:].ap[0], [0, K], [1, B]])
            nc.vector.tensor_mul(y_g[:], eTrep[g][:], xb)
            for grp in range(4):
                nc.tensor.matmul(ps_s2[grp][:],
                                 lhsT=wt_sb[g][:, 128 * grp:128 * (grp + 1)],
                                 rhs=y_g[:, 512 * grp:512 * (grp + 1)],
                                 start=(g == 0), stop=(g == G - 1))
        for grp in range(4):
            s2f = sm.tile([128, 512], F32, tag="s2f")
            nc.scalar.copy(s2f[:], ps_s2[grp][:])
            for k8 in range(8):
                dst = _ap(ar2_in[:], (16 * k8) * 256 + 64 * grp,
                          [[256, 16], [1, 64]])
                nc.sync.dma_start(
                    out=dst,
                    in_=s2f[16 * k8:16 * (k8 + 1), 64 * k8:64 * (k8 + 1)])
    nc.gpsimd.collective_compute(
        "AllReduce", mybir.AluOpType.add,
        replica_groups=[list(range(NC))], ins=[ar2_in.opt()],
        outs=[ar2_out.opt()])

    # ---------------- tail: transpose + squash (all k; host picks) --------
    rs_sb = sm.tile([128, 256], F32, tag="rs")
    nc.sync.dma_start(out=rs_sb[:], in_=ar2_out[:])
    v2T = sm.tile([128, 256], F16, tag="v2T")
    nc.scalar.copy(v2T[:], rs_sb[:])
    for h in range(2):
        pt = psT.tile([128, 128], F16, tag="psTt")
        nc.tensor.transpose(pt[:], v2T[:, h * 128:(h + 1) * 128], idn_sb[:])
        s2b = sm.tile([128, 128], F32, tag="s2b")
        nc.scalar.copy(s2b[:], pt[:])
        s3b = s2b[:].rearrange("p (k i) -> p k i", i=I)
        ss = sm.tile([128, 8, I], F32, tag="ss")
        nc.vector.tensor_mul(ss[:], s3b, s3b)
        nn = sm.tile([128, 8], F32, tag="nn")
        nc.vector.reduce_sum(nn[:], ss[:], axis=mybir.AxisListType.X)
        rt2 = sm.tile([128, 8], F32, tag="rt2")
        nc.scalar.activation(rt2[:], nn[:],
                             mybir.ActivationFunctionType.Sqrt,
                             bias=eps_p[:], scale=1.0)
        dn2 = sm.tile([128, 8], F32, tag="dn2")
        nc.scalar.add(dn2[:], nn[:], 1.0)
        dd2 = sm.tile([128, 8], F32, tag="dd2")
        nc.vector.tensor_mul(dd2[:], dn2[:], rt2[:])
        rc2 = sm.tile([128, 8], F32, tag="rc2")
        nc.vector.reciprocal(rc2[:], dd2[:])
        f2 = sm.tile([128, 8], F32, tag="f2")
        nc.vector.tensor_mul(f2[:], nn[:], rc2[:])
        v2 = sm.tile([128, 8, I], F32, tag="v2")
        nc.vector.tensor_mul(v2[:], s3b, _bcast(f2[:], I))
        nc.sync.dma_start(
            out=_ap(out_d, h * 128 * 128, [[128, 128], [1, 128]]),
            in_=v2[:].rearrange("p k i -> p (k i)"))


_PROG = None


def _get_program():
    global _PROG
    if _PROG is None:
        nc = bacc.Bacc("TRN2", target_bir_lowering=False, debug=False,
                       num_devices=NC)
        wt_d = nc.dram_tensor("wt", [CL * J, KI], F16, kind="ExternalInput")
        w2_d = nc.dram_tensor("w2", [4, 128, 2048], F16, kind="ExternalInput")
        xcb_d = nc.dram_tensor("xcb", [CL * J, B], F16, kind="ExternalInput")
        xrep_d = nc.dram_tensor("xrep", [128, 2048], F16,
                                kind="ExternalInput")
        sel2_d = nc.dram_tensor("sel2", [128, B], F16, kind="ExternalInput")
        idn_d = nc.dram_tensor("idn", [128, 128], F16, kind="ExternalInput")
        idn32_d = nc.dram_tensor("idn32", [16, 16], F32, kind="ExternalInput")
        mask4_d = nc.dram_tensor("mask4", [128, 512], F16,
                                 kind="ExternalInput")
        out_d = nc.dram_tensor("out", [256, 128], F32, kind="ExternalOutput")
        with tile.TileContext(nc) as tc:
            _body(tc, wt_d[:], w2_d[:], xcb_d[:], xrep_d[:], sel2_d[:],
                  idn_d[:], idn32_d[:], mask4_d[:], out_d[:])
        nc.compile()
        _PROG = nc
    return _PROG


def _consts():
    sel2 = np.zeros((2, B, B), np.float16)
    for b in range(B):
        sel2[:, b, b] = 1.0
    sel2 = sel2.reshape(128, B)
    idn = np.eye(128, dtype=np.float16)
    idn32 = np.eye(16, dtype=np.float32)
    # [p2r, k2, i, p2c, k2', b]: diagonal in BOTH p2 and k2
    mask4 = np.zeros((4, 2, 16, 4, 2, B), np.float16)
    for p2 in range(4):
        for k2 in range(2):
            mask4[p2, k2, :, p2, k2, :] = 1.0
    mask4 = mask4.reshape(128, 512)
    return sel2, idn, idn32, mask4


def kernel(x, W):
    global LAST_RESULTS
    x = np.ascontiguousarray(np.asarray(x, np.float32))
    W = np.ascontiguousarray(np.asarray(W, np.float32))
    assert x.shape == (B, C, J) and W.shape == (K, C, I, J)
    nc = _get_program()
    sel2, idn, idn32, mask4 = _consts()
    in_maps = []
    for m in range(NC):
        xs = x[:, m * CL:(m + 1) * CL, :]                  # [B, CL, J]
        Ws = W[:, m * CL:(m + 1) * CL, :, :]               # [K, CL, I, J]
        wt = np.ascontiguousarray(
            Ws.transpose(1, 3, 0, 2)).reshape(CL * J, KI).astype(np.float16)
        # w2: rows (k,i) k-major in 4 blocks of 128; cols (j,c) j-major
        w2 = np.ascontiguousarray(
            Ws.transpose(0, 2, 3, 1)).reshape(KI, J * CL)
        w2 = w2.reshape(4, 128, J * CL).astype(np.float16)
        xcb = np.ascontiguousarray(
            xs.transpose(1, 2, 0)).reshape(CL * J, B).astype(np.float16)
        # xrep: rows (k2,b), cols (j,c) j-major
        xj = np.ascontiguousarray(xs.transpose(0, 2, 1)).reshape(B, J * CL)
        xrep = np.tile(xj, (2, 1)).astype(np.float16)
        in_maps.append({"wt": wt, "w2": w2, "xcb": xcb, "xrep": xrep,
                        "sel2": sel2, "idn": idn, "idn32": idn32,
                        "mask4": mask4})
    res = run_bass_kernel_spmd(nc, in_maps, core_ids=list(range(NC)),
                               trace=TRACE)
    LAST_RESULTS = res
    # core 0 output: rows (grp,b), cols (k8,i)
    om = np.asarray(res.results[0]["out"], np.float32).reshape(4, B, 8, I)
    out = np.empty((B, K, I), np.float32)
    for grp in range(4):
        for k8 in range(8):
            out[:, 8 * grp + k8, :] = om[grp, :, k8, :]
    return out


# revision 14
# speedup vs baseline: 1.3364x; 1.2011x over previous
"""DenseCapsule routing (2 iterations) on 8 Trainium2 cores — P-route.

Sharding: caps_in (C=2048) split across 8 cores (256 each).

Math (ITERATIONS=2, v0=0 => logits after iter1 are 0, cc1 = 1/K):
  s1[b,ki]   = sum_{c,j} W x          (pass1 matmul, AllReduce)
  v1         = squash(s1/K)
  P[b,k,c,j] = sum_i v1[b,k,i] W[k,c,i,j]     (PE: v1 block-diag x W2)
  a[b,k,c]   = sum_j x[b,c,j] P[b,k,c,j]      (DVE mult + j-tree, lvl1 Pool)
  e = exp(a);  den[b,c] = sum_k e             (PE: sel2 matmul)
  xden[(c,j),b] = x/den;  y[(c,j),(k,b)] = eT * xden   (DVE)
  s2T[(k8,i),(grp,b)] = diag_k8( wt_grp^T y_grp )      (PE + diag DMAs)
  AllReduce(s2T); squash; host reassembles k-order.

Layouts (free dims j-major for the a-branch, c-major rows for s2):
  wt   [(c,j)=2048 c-major rows, (k,i)=512]   pass1 rhs + s2 lhsT
  w2   [4][128=(p2,k2,i), (j,c)=2048 j-major] P rhs (4 pairs per tile)
  xcb  [(c,j)=2048 c-major, b=64]             pass1 lhsT + xden src
  xrep [128=(k2,b), (j,c)=2048 j-major]       a-branch mult operand
  sel2 [128=(k2,b), 64]  delta(b'=b)          den matmul lhsT
"""

from contextlib import ExitStack

import numpy as np

import concourse.bacc as bacc
import concourse.bass as bass
import concourse.tile as tile
from concourse import mybir
from concourse._compat import with_exitstack
from concourse.bass_utils import run_bass_kernel_spmd

NC = 8
B = 64
C = 2048
J = 8
K = 32
I = 16
CL = C // NC        # 256 local caps_in
G = CL // 16        # 16 (c,j)-row tiles of 128
KI = K * I          # 512
NP = K // 2         # 16 k-pairs
EPS = 1e-7

F32 = mybir.dt.float32
F16 = mybir.dt.float16

TRACE = False
LAST_RESULTS = None


def _ap(t_ap, off, dims):
    return bass.AP(tensor=t_ap.tensor, offset=t_ap.offset + off, ap=dims)


def _bcast(ap, n):
    """Append a stride-0 dim of size n (free-dim broadcast)."""
    return bass.AP(tensor=ap.tensor, offset=ap.offset, ap=[*ap.ap, [0, n]])


@with_exitstack
def _body(ctx, tc, wt, w2, xcb, xrep, sel2, idn, idn32, mask4, out_d):
    nc = tc.nc
    singles = ctx.enter_context(tc.tile_pool(name="singles", bufs=1))
    sm = ctx.enter_context(tc.tile_pool(name="sm", bufs=2))
    amp = ctx.enter_context(tc.tile_pool(name="amp", bufs=2))
    ypool = ctx.enter_context(tc.tile_pool(name="ypool", bufs=3))
    psT = ctx.enter_context(tc.tile_pool(name="psT", bufs=2, space="PSUM"))
    dram = ctx.enter_context(tc.tile_pool(name="dram", bufs=1, space="DRAM"))

    ar1_in = dram.tile([B, KI], F32, name="ar1_in")
    ar1_out = dram.tile([B, KI], F32, name="ar1_out", addr_space="Shared")
    eTd = dram.tile([CL, 4 * KI], F16, name="eTd")
    rdTd = dram.tile([CL, B], F16, name="rdTd")
    s2d = dram.tile([4, 128, 512], F32, name="s2d")
    ar2_in = dram.tile([128, 256], F32, name="ar2_in")
    ar2_out = dram.tile([128, 256], F32, name="ar2_out", addr_space="Shared")

    # ------------- resident inputs (few big DMAs; wt/xcb first) -------------
    wt_sb = singles.tile([128, G, KI], F16, name="wt_sb")
    xcb_sb = singles.tile([128, G, B], F16, name="xcb_sb")
    w2_sb = singles.tile([128, 4, 2048], F16, name="w2_sb")
    xrep_sb = singles.tile([128, 2048], F16, name="xrep_sb")
    sel2_sb = singles.tile([128, B], F16, name="sel2_sb")
    idn_sb = singles.tile([128, 128], F16, name="idn_sb")
    idn32_sb = singles.tile([16, 16], F32, name="idn32_sb")
    mask4_sb = singles.tile([128, 512], F16, name="mask4_sb")
    nc.sync.dma_start(out=wt_sb[:],
                      in_=_ap(wt, 0, [[KI, 128], [KI * 128, G], [1, KI]]))
    nc.sync.dma_start(out=xcb_sb[:],
                      in_=_ap(xcb, 0, [[B, 128], [B * 128, G], [1, B]]))
    nc.scalar.dma_start(
        out=w2_sb[:],
        in_=_ap(w2, 0, [[2048, 128], [2048 * 128, 4], [1, 2048]]))
    nc.scalar.dma_start(out=xrep_sb[:], in_=xrep)
    nc.scalar.dma_start(out=sel2_sb[:], in_=sel2)
    nc.scalar.dma_start(out=idn_sb[:], in_=idn)
    nc.scalar.dma_start(out=idn32_sb[:], in_=idn32)
    nc.scalar.dma_start(out=mask4_sb[:], in_=mask4)

    eps_t = singles.tile([B, 1], F32)
    nc.vector.memset(eps_t[:], EPS)
    eps_p = singles.tile([128, 1], F32)
    nc.vector.memset(eps_p[:], EPS)

    # v1 block-diag (filled via mask multiply after v1T is ready)
    v1bd = singles.tile([128, NP * 128], F16, name="v1bd")

    # ---------------- pass 1 + AllReduce ----------------
    with tc.tile_pool(name="psP1", bufs=1, space="PSUM") as psP1:
        ps_s1 = psP1.tile([B, KI], F32)
        for g in range(G):
            nc.tensor.matmul(ps_s1[:], lhsT=xcb_sb[:, g, :],
                             rhs=wt_sb[:, g, :],
                             start=(g == 0), stop=(g == G - 1))
        s1p = sm.tile([B, KI], F32, tag="s1p")
        nc.scalar.copy(s1p[:], ps_s1[:])
    nc.sync.dma_start(out=ar1_in[:], in_=s1p[:])
    nc.gpsimd.collective_compute(
        "AllReduce", mybir.AluOpType.add,
        replica_groups=[list(range(NC))], ins=[ar1_in.opt()],
        outs=[ar1_out.opt()])
    s1 = sm.tile([B, KI], F32, tag="s1")
    nc.sync.dma_start(out=s1[:], in_=ar1_out[:])

    # ---------------- squash(s1/K) -> v1b fp16 ----------------
    pre = 1.0 / K
    s3 = s1[:].rearrange("p (k i) -> p k i", i=I)
    sq = sm.tile([B, K, I], F32, tag="sq")
    nc.vector.tensor_mul(sq[:], s3, s3)
    n0 = sm.tile([B, K], F32, tag="n0")
    nc.vector.reduce_sum(n0[:], sq[:], axis=mybir.AxisListType.X)
    sn = sm.tile([B, K], F32, tag="sn")
    nc.scalar.mul(sn[:], n0[:], pre * pre)
    rt = sm.tile([B, K], F32, tag="rt")
    nc.scalar.activation(rt[:], sn[:], mybir.ActivationFunctionType.Sqrt,
                         bias=eps_t[:], scale=1.0)
    dn = sm.tile([B, K], F32, tag="dn")
    nc.scalar.add(dn[:], sn[:], 1.0)
    dd = sm.tile([B, K], F32, tag="dd")
    nc.vector.tensor_mul(dd[:], dn[:], rt[:])
    rc = sm.tile([B, K], F32, tag="rc")
    nc.vector.reciprocal(rc[:], dd[:])
    f0 = sm.tile([B, K], F32, tag="f0")
    nc.vector.tensor_mul(f0[:], n0[:], rc[:])
    g0 = sm.tile([B, K], F32, tag="g0")
    nc.scalar.mul(g0[:], f0[:], pre * pre * pre)
    v1b = sm.tile([B, K, I], F16, tag="v1b")
    nc.vector.tensor_mul(v1b[:], s3, _bcast(g0[:], I))

    # ---------------- v1T via PE transpose; fill v1bd ----------------
    v1T = [singles.tile([128, B], F16, name=f"v1T{t}") for t in range(4)]
    v1f = v1b[:].rearrange("p k i -> p (k i)")
    for t in range(4):
        pt = psT.tile([128, 128], F16, tag="psTh")
        nc.tensor.transpose(pt[:, :B], v1f[:, t * 128:(t + 1) * 128],
                            idn_sb[:B, :B])
        nc.scalar.copy(v1T[t][:], pt[:, :B])
    for t in range(4):
        src0 = bass.AP(tensor=v1T[t][:].tensor, offset=v1T[t][:].offset,
                       ap=[v1T[t][:].ap[0], [0, 8], [1, B]])
        src1 = bass.AP(tensor=mask4_sb[:].tensor, offset=mask4_sb[:].offset,
                       ap=[mask4_sb[:].ap[0], [B, 8], [1, B]])
        dst = v1bd[:, 512 * t:512 * (t + 1)].rearrange(
            "p (q b) -> p q b", b=B)
        nc.vector.tensor_mul(dst, src0, src1)

    # ---------------- phase A: per-pair P -> a -> e ----------------
    eT2c = singles.tile([128, 2, 4 * KI], F16, name="eT2c")
    eTrep = [singles.tile([128, 2048], F16, name=f"eTrep{g}")
             for g in range(G)]

    with ExitStack() as aes:
        psP = aes.enter_context(
            tc.tile_pool(name="psP", bufs=3, space="PSUM"))
        psDen = aes.enter_context(
            tc.tile_pool(name="psDen", bufs=1, space="PSUM"))
        ps_den = psDen.tile([B, CL], F32)

        def phaseA(p):
            t = p // 4
            Pb = amp.tile([128, 1024], F16, tag="Pb")     # ACT-drained halves
            am = amp.tile([128, 2048], F16, tag="am")
            for q in range(4):
                ps_q = psP.tile([128, 512], F32, tag="psq")
                nc.tensor.matmul(ps_q[:], lhsT=v1bd[:, 128 * p:128 * (p + 1)],
                                 rhs=w2_sb[:, t, q * 512:(q + 1) * 512],
                                 start=True, stop=True)
                sl = slice(q * 512, (q + 1) * 512)
                if q % 2 == 0:
                    nc.scalar.copy(Pb[:, (q // 2) * 512:(q // 2 + 1) * 512],
                                   ps_q[:])
                else:
                    nc.vector.tensor_mul(am[:, sl], ps_q[:], xrep_sb[:, sl])
            for q in (0, 2):
                sl = slice(q * 512, (q + 1) * 512)
                nc.vector.tensor_mul(
                    am[:, sl], Pb[:, (q // 2) * 512:(q // 2 + 1) * 512],
                    xrep_sb[:, sl])
            a3 = am[:].rearrange("p (j c) -> p j c", c=CL)
            t1 = amp.tile([128, 4, CL], F16, tag="t1")
            nc.gpsimd.tensor_add(t1[:], a3[:, 0:4, :], a3[:, 4:8, :])
            t2 = amp.tile([128, 2, CL], F16, tag="t2")
            nc.vector.tensor_add(t2[:], t1[:, 0:2, :], t1[:, 2:4, :])
            a_p = amp.tile([128, CL], F16, tag="a")
            nc.vector.tensor_add(a_p[:], t2[:, 0, :], t2[:, 1, :])
            e_p = amp.tile([128, CL], F16, tag="e")
            nc.scalar.activation(e_p[:], a_p[:],
                                 mybir.ActivationFunctionType.Exp, scale=1.0)
            # den accumulation over pairs
            nc.tensor.matmul(ps_den[:], lhsT=sel2_sb[:], rhs=e_p[:],
                             start=(p == 0), stop=(p == NP - 1))
            # eT: transpose e_p halves into eT2c; one staging DMA to eTd
            for h in range(2):
                pt = psT.tile([128, 128], F16, tag="psTh")
                nc.tensor.transpose(pt[:], e_p[:, h * 128:(h + 1) * 128],
                                    idn_sb[:])
                nc.scalar.copy(eT2c[:, h, 128 * p:128 * (p + 1)], pt[:])
            nc.scalar.dma_start(
                out=_ap(eTd[:], 128 * p,
                        [[4 * KI, 128], [128 * 4 * KI, 2], [1, 128]]),
                in_=eT2c[:, :, 128 * p:128 * (p + 1)])

        def rep_dmas(gp):
            # replicate eTd rows (c) x8 over j -> eTrep[g][:, gp half]
            for g in range(G):
                src = _ap(eTd[:], (16 * g) * (4 * KI) + 1024 * gp,
                          [[4 * KI, 16], [0, 8], [1, 1024]])
                nc.sync.dma_start(
                    out=eTrep[g][:, 1024 * gp:1024 * (gp + 1)], in_=src)

        for p in range(NP):
            phaseA(p)
            if p % 8 == 7:
                rep_dmas(p // 8)

        # ---------------- den -> xden ----------------
        rcd = sm.tile([B, CL], F32, tag="rcd")
        nc.vector.reciprocal(rcd[:], ps_den[:])
    rch = sm.tile([B, CL], F16, tag="rch")
    nc.scalar.copy(rch[:], rcd[:])
    rdT = [sm.tile([128, B], F16, name=f"rdT{h}") for h in range(2)]
    for h in range(2):
        pt = psT.tile([128, 128], F16, tag="psTh")
        nc.tensor.transpose(pt[:, :B], rch[:, h * 128:(h + 1) * 128],
                            idn_sb[:B, :B])
        nc.scalar.copy(rdT[h][:], pt[:, :B])
        nc.scalar.dma_start(out=rdTd[h * 128:(h + 1) * 128, :],
                            in_=rdT[h][:])
    xden = [singles.tile([128, B], F16, name=f"xden{g}") for g in range(G)]
    xdrep = [sm.tile([128, B], F16, name=f"xdr{g}") for g in range(G)]
    for g in range(G):
        src = _ap(rdTd[:], (16 * g) * B, [[B, 16], [0, 8], [1, B]])
        nc.scalar.dma_start(out=xdrep[g][:], in_=src)
        nc.gpsimd.tensor_mul(xden[g][:], xcb_sb[:, g, :], xdrep[g][:])

    # ---------------- phase B: y, s2 matmuls, diag extract ----------------
    with tc.tile_pool(name="psS2", bufs=1, space="PSUM") as psS2:
        ps_s2 = [psS2.tile([128, 512], F32, name=f"pss2{grp}")
                 for grp in range(4)]

        def finish_grp(grp):
            s2f = sm.tile([128, 512], F32, tag=f"s2f{grp % 2}")
            nc.scalar.copy(s2f[:], ps_s2[grp][:])
            nc.sync.dma_start(out=s2d[grp], in_=s2f[:])
            src = _ap(s2d[:], grp * 128 * 512,
                      [[16 * 512 + 64, 8], [512, 16], [1, 64]])
            dst = _ap(ar2_in[:], 64 * grp,
                      [[16 * 256, 8], [256, 16], [1, 64]])
            nc.sync.dma_start(out=dst, in_=src)

        for gp in range(2):
            for g in range(G):
                y_gg = ypool.tile([128, 1024], F16, tag=f"y{gp}")
                xb = bass.AP(tensor=xden[g][:].tensor,
                             offset=xden[g][:].offset,
                             ap=[xden[g][:].ap[0], [0, 16], [1, B]])
                nc.vector.tensor_mul(
                    y_gg[:], eTrep[g][:, 1024 * gp:1024 * (gp + 1)], xb)
                for q2 in range(2):
                    grp = 2 * gp + q2
                    nc.tensor.matmul(
                        ps_s2[grp][:],
                        lhsT=wt_sb[:, g, 128 * grp:128 * (grp + 1)],
                        rhs=y_gg[:, 512 * q2:512 * (q2 + 1)],
                        start=(g == 0), stop=(g == G - 1))
            finish_grp(2 * gp)
            finish_grp(2 * gp + 1)

    nc.gpsimd.collective_compute(
        "AllReduce", mybir.AluOpType.add,
        replica_groups=[list(range(NC))], ins=[ar2_in.opt()],
        outs=[ar2_out.opt()])

    # ---------------- tail: transpose + squash (all k; host picks) --------
    rs_sb = sm.tile([128, 256], F32, tag="rs")
    nc.sync.dma_start(out=rs_sb[:], in_=ar2_out[:])
    v2T = sm.tile([128, 256], F16, tag="v2T")
    nc.scalar.copy(v2T[:], rs_sb[:])
    for h in range(2):
        pt = psT.tile([128, 128], F16, tag="psTt")
        nc.tensor.transpose(pt[:], v2T[:, h * 128:(h + 1) * 128], idn_sb[:])
        s2b = sm.tile([128, 128], F32, tag="s2b")
        nc.scalar.copy(s2b[:], pt[:])
        s3b = s2b[:].rearrange("p (k i) -> p k i", i=I)
        ss = sm.tile([128, 8, I], F32, tag="ss")
        nc.vector.tensor_mul(ss[:], s3b, s3b)
        nn = sm.tile([128, 8], F32, tag="nn")
        nc.vector.reduce_sum(nn[:], ss[:], axis=mybir.AxisListType.X)
        rt2 = sm.tile([128, 8], F32, tag="rt2")
        nc.scalar.activation(rt2[:], nn[:],
                             mybir.ActivationFunctionType.Sqrt,
                             bias=eps_p[:], scale=1.0)
        dn2 = sm.tile([128, 8], F32, tag="dn2")
        nc.scalar.add(dn2[:], nn[:], 1.0)
        dd2 = sm.tile([128, 8], F32, tag="dd2")
        nc.vector.tensor_mul(dd2[:], dn2[:], rt2[:])
        rc2 = sm.tile([128, 8], F32, tag="rc2")
        nc.vector.reciprocal(rc2[:], dd2[:])
        f2 = sm.tile([128, 8], F32, tag="f2")
        nc.vector.tensor_mul(f2[:], nn[:], rc2[:])
        v2 = sm.tile([128, 8, I], F32, tag="v2")
        nc.vector.tensor_mul(v2[:], s3b, _bcast(f2[:], I))
        nc.sync.dma_start(
            out=_ap(out_d, h * 128 * 128, [[128, 128], [1, 128]]),
            in_=v2[:].rearrange("p k i -> p (k i)"))


_PROG = None


def _get_program():
    global _PROG
    if _PROG is None:
        nc = bacc.Bacc("TRN2", target_bir_lowering=False, debug=False,
                       num_devices=NC)
        wt_d = nc.dram_tensor("wt", [CL * J, KI], F16, kind="ExternalInput")
        w2_d = nc.dram_tensor("w2", [4, 128, 2048], F16, kind="ExternalInput")
        xcb_d = nc.dram_tensor("xcb", [CL * J, B], F16, kind="ExternalInput")
        xrep_d = nc.dram_tensor("xrep", [128, 2048], F16,
                                kind="ExternalInput")
        sel2_d = nc.dram_tensor("sel2", [128, B], F16, kind="ExternalInput")
        idn_d = nc.dram_tensor("idn", [128, 128], F16, kind="ExternalInput")
        idn32_d = nc.dram_tensor("idn32", [16, 16], F32, kind="ExternalInput")
        mask4_d = nc.dram_tensor("mask4", [128, 512], F16,
                                 kind="ExternalInput")
        out_d = nc.dram_tensor("out", [256, 128], F32, kind="ExternalOutput")
        with tile.TileContext(nc) as tc:
            _body(tc, wt_d[:], w2_d[:], xcb_d[:], xrep_d[:], sel2_d[:],
                  idn_d[:], idn32_d[:], mask4_d[:], out_d[:])
        nc.compile()
        _PROG = nc
    return _PROG


def _consts():
    sel2 = np.zeros((2, B, B), np.float16)
    for b in range(B):
        sel2[:, b, b] = 1.0
    sel2 = sel2.reshape(128, B)
    idn = np.eye(128, dtype=np.float16)
    idn32 = np.eye(16, dtype=np.float32)
    # [p2r, k2, i, p2c, k2', b]: diagonal in BOTH p2 and k2
    mask4 = np.zeros((4, 2, 16, 4, 2, B), np.float16)
    for p2 in range(4):
        for k2 in range(2):
            mask4[p2, k2, :, p2, k2, :] = 1.0
    mask4 = mask4.reshape(128, 512)
    return sel2, idn, idn32, mask4


def kernel(x, W):
    global LAST_RESULTS
    x = np.ascontiguousarray(np.asarray(x, np.float32))
    W = np.ascontiguousarray(np.asarray(W, np.float32))
    assert x.shape == (B, C, J) and W.shape == (K, C, I, J)
    nc = _get_program()
    sel2, idn, idn32, mask4 = _consts()
    in_maps = []
    for m in range(NC):
        xs = x[:, m * CL:(m + 1) * CL, :]                  # [B, CL, J]
        Ws = W[:, m * CL:(m + 1) * CL, :, :]               # [K, CL, I, J]
        wt = np.ascontiguousarray(
            Ws.transpose(1, 3, 0, 2)).reshape(CL * J, KI).astype(np.float16)
        # w2: rows (k,i) k-major in 4 blocks of 128; cols (j,c) j-major
        w2 = np.ascontiguousarray(
            Ws.transpose(0, 2, 3, 1)).reshape(KI, J * CL)
        w2 = w2.reshape(4, 128, J * CL).astype(np.float16)
        xcb = np.ascontiguousarray(
            xs.transpose(1, 2, 0)).reshape(CL * J, B).astype(np.float16)
        # xrep: rows (k2,b), cols (j,c) j-major
        xj = np.ascontiguousarray(xs.transpose(0, 2, 1)).reshape(B, J * CL)
        xrep = np.tile(xj, (2, 1)).astype(np.float16)
        in_maps.append({"wt": wt, "w2": w2, "xcb": xcb, "xrep": xrep,
                        "sel2": sel2, "idn": idn, "idn32": idn32,
                        "mask4": mask4})
    res = run_bass_kernel_spmd(nc, in_maps, core_ids=list(range(NC)),
                               trace=TRACE)
    LAST_RESULTS = res
    # core 0 output: rows (grp,b), cols (k8,i)
    om = np.asarray(res.results[0]["out"], np.float32).reshape(4, B, 8, I)
    out = np.empty((B, K, I), np.float32)
    for grp in range(4):
        for k8 in range(8):
            out[:, 8 * grp + k8, :] = om[grp, :, k8, :]
    return out
